# revision 57
# baseline (speedup 1.0000x reference)
"""DilateAttention Trainium2 Bass kernel.

Problem: q,k,v [16, 128, 64, 64] f32; per-pixel attention over 9 dilated
(dil=2) 3x3 neighbors per head (4 heads x 32 dim); out [16, 64, 64, 128].

Sharding: data-parallel over batch B across 8 cores (2 images/core).

Layout: channel-major ([128 ch partitions, pixels free]). K and V are kept
as zero-padded 68x68 bf16 images so every shifted neighbor view is a
regular (dx, row, col) access pattern; the zero padding reproduces torch
Unfold semantics exactly, including the exp(0) softmax denominator terms
at borders.

Per 4-row chunk (256 px), kk grouped by dy (3 groups of 3), software
pipelined DEPTH=6 chunks deep:
  QK products Q*K_kk (DVE bf16 tensor_tensor, 2x_1p mode, one op per dy
  group with a broadcast-Q AP) -> per-head score sums via PE block-ones
  matmuls into grouped PSUM tiles ({2,1}-plane bank windows) -> exp on
  ACT (PSUM -> SBUF bf16, one op per dy group, written into a single
  9-plane E tile; run at full 128-partition extent so the per-head ->
  per-channel broadcast comes out of the exp for free) -> denominator via
  9 accumulated PE (block-ones/32) matmuls -> AV products E*V (DVE for dy
  groups 0-1, Pool for group 2) -> kk-sum via 9 accumulated PE identity
  matmuls -> reciprocal_approx_fast + normalize multiply (DVE; PSUM is
  DVE/ACT-only and a DVE op may read at most one PSUM operand) ->
  channel-major bf16 store straight from SBUF.  The host gather upcasts
  to f32 and transposes [B,D,H,W] -> [B,H,W,D] (layout only; all
  arithmetic happens on device).

Per-image prep: q/k/v DMA'd in row bands (6/10/16/16/16 rows) to f32
band stages, cast to bf16 on Pool (q/k/v; head bands fan out across
DVE/ACT/Pool), k/v strided into the padded 68x68 images whose borders
are zeroed by 3 merged Pool memsets.  The next image's load/cast ops are
dripped between chunk emissions so the in-order engine queues never
stall a chunk behind prep work; image 0's late bands drip into its own
first chunk slots the same way.

Hardware constraints found the hard way (the cost model accepts all of
these; the BIR verifier / codegen rejects them):
  - GPSIMD (Pool) cannot access PSUM at all.
  - DVE ops may read at most ONE non-scalar operand from PSUM.
  - tensor_tensor ALU has no divide; reciprocal_approx_* requires f32.
  - TensorTensor APs are capped at 3 free dims (so products go per dy
    group, not as one 9-plane op).
  - ScalarTensorTensor (TensorScalarPtr) is not supported on Pool and
    requires <= 3D operands on DVE.

Engine busy (timeline cost model, per core): DVE ~100us (products,
recip, normalize), PE ~95us (scores 9N + Z 9N + Oacc 9N cols/chunk),
ACT ~89us (exp + ramp casts), Pool ~72us (casts, AV tail, memsets);
wall 109.6us.
"""

import numpy as np
from contextlib import ExitStack

import concourse.bass as bass
import concourse.bacc as bacc
import concourse.tile as tile
from concourse import mybir
from concourse.bass_utils import run_bass_kernel_spmd
from concourse.masks import make_identity

F32 = mybir.dt.float32
BF16 = mybir.dt.bfloat16

ALU = mybir.AluOpType

B, D, H, W = 16, 128, 64, 64
NCORES = 8
BLOC = B // NCORES          # images per core
HEADS, HD = 4, 32
KS, DIL, PAD = 3, 2, 2
HP = H + 2 * PAD            # 68 (y-padded)
WP2 = W + 2 * PAD           # 68 (x-padded)
KK = KS * KS                # 9
SCALE = float(HD) ** -0.5
R = 4                       # image rows per chunk
NC = R * W                  # 256 pixels per chunk
NCHUNK = H // R             # 16
OGRP = 2                    # chunks batched per output DMA

BANDS = [(0, 6), (6, 10), (16, 16), (32, 16), (48, 16)]  # (row0, nrows)

PROFILE = False


def _bcast_q(qbf, y0):
    """[128, 3, R, W] view of qbf rows y0..y0+R with a 0-step dx axis."""
    base = qbf[:, y0:y0 + R, :]
    return bass.AP(
        tensor=base.tensor,
        offset=base.offset,
        ap=[list(base.ap[0]), [0, KS], [W, R], [1, W]],
    )


def _shift_view(pad_t, y0, idy, i0=0, n=KS):
    """[128, n(idx), R, W] view of padded image at dy=idy for chunk y0,
    idx range [i0, i0+n)."""
    return bass.AP(
        tensor=pad_t.tensor,
        offset=pad_t.offset + (y0 + DIL * idy) * WP2 + DIL * i0,
        ap=[list(pad_t.ap[0]), [DIL, n], [WP2, R], [1, W]],
    )


def _pad_views(pad_t):
    """Three memset APs covering the zero border of a [128, HP, WP2] image:
    top rows + row-2 left pad (contiguous), the interleaved right/left pad
    strip, and row-65 right pad + bottom rows (contiguous)."""
    p0 = list(pad_t.ap[0])
    top = bass.AP(tensor=pad_t.tensor, offset=pad_t.offset,
                  ap=[p0, [1, PAD * WP2 + PAD]])
    mid = bass.AP(tensor=pad_t.tensor,
                  offset=pad_t.offset + PAD * WP2 + WP2 - PAD,
                  ap=[p0, [WP2, H - 1], [1, 2 * PAD]])
    bot = bass.AP(tensor=pad_t.tensor,
                  offset=pad_t.offset + (HP - PAD) * WP2 - PAD,
                  ap=[p0, [1, PAD * WP2 + PAD]])
    return top, mid, bot


def _body(ctx: ExitStack, tc: tile.TileContext, out_ap, q_ap, k_ap, v_ap):
    nc = tc.nc

    consts = ctx.enter_context(tc.tile_pool(name="consts", bufs=1))
    stage_pool = ctx.enter_context(tc.tile_pool(name="stage_pool", bufs=6))
    perb = ctx.enter_context(tc.tile_pool(name="perb", bufs=2))
    egrp = ctx.enter_context(tc.tile_pool(name="egrp", bufs=4))
    epool = ctx.enter_context(tc.tile_pool(name="epool", bufs=8))
    work = ctx.enter_context(tc.tile_pool(name="work", bufs=8))
    psS = ctx.enter_context(tc.tile_pool(name="psS", bufs=2, space="PSUM"))
    psZ = ctx.enter_context(tc.tile_pool(name="psZ", bufs=4, space="PSUM"))

    # Constant stationary matrices (emitted after the first band DMAs are
    # queued so the setup memsets overlap the transfers).
    blockones = consts.tile([128, 128], BF16)   # 1 if same head
    bo32 = consts.tile([128, 128], BF16)        # 1/32 if same head
    identb = consts.tile([128, 128], BF16)

    def emit_consts():
        nc.vector.memset(blockones, 0.0)
        for h in range(HEADS):
            s = slice(h * HD, (h + 1) * HD)
            nc.vector.memset(blockones[s, s], 1.0)
        nc.vector.memset(bo32, 0.0)
        for h in range(HEADS):
            s = slice(h * HD, (h + 1) * HD)
            nc.vector.memset(bo32[s, s], 1.0 / HD)
        make_identity(nc, identb)

    out_flat = out_ap.rearrange("b d h w -> b d (h w)")

    # Per-image padded bf16 images, filled band by band via the dripped
    # prep schedule below.
    imgs = {}

    def prep_alloc(b, dma_first=False):
        qbf = perb.tile([128, H, W], BF16, name="qbf")
        kpad = perb.tile([128, HP, WP2], BF16, name="kpad")
        vpad = perb.tile([128, HP, WP2], BF16, name="vpad")
        imgs[b] = (qbf, kpad, vpad)
        if dma_first:
            prep_dma(b, 0, kq_only=(b == 0))
        for t in (kpad, vpad):
            top, mid, bot = _pad_views(t)
            nc.gpsimd.memset(top, 0.0)
            nc.gpsimd.memset(mid, 0.0)
            nc.gpsimd.memset(bot, 0.0)

    def prep_dma(b, j, kq_only=False):
        r0, nr = BANDS[j]
        rs = slice(r0, r0 + nr)
        kst = stage_pool.tile([128, nr, W], F32, name=f"kst{nr}", tag=f"s{nr}")
        vst = stage_pool.tile([128, nr, W], F32, name=f"vst{nr}", tag=f"s{nr}")
        qst = stage_pool.tile([128, nr, W], F32, name=f"qst{nr}", tag=f"s{nr}")
        nc.sync.dma_start(out=kst, in_=k_ap[b][:, rs, :])
        nc.sync.dma_start(out=qst, in_=q_ap[b][:, rs, :])
        if not kq_only:
            nc.sync.dma_start(out=vst, in_=v_ap[b][:, rs, :])
        imgs.setdefault("st", {})[(b, j)] = (qst, kst, vst)

    def prep_dma_v(b, j):
        r0, nr = BANDS[j]
        rs = slice(r0, r0 + nr)
        _, _, vst = imgs["st"][(b, j)]
        nc.sync.dma_start(out=vst, in_=v_ap[b][:, rs, :])

    def prep_cast(b, j, spread=False):
        qbf, kpad, vpad = imgs[b]
        qst, kst, vst = imgs["st"].pop((b, j))
        r0, nr = BANDS[j]
        rs = slice(r0, r0 + nr)
        kdst = kpad[:, PAD + r0:PAD + r0 + nr, PAD:WP2 - PAD]
        vdst = vpad[:, PAD + r0:PAD + r0 + nr, PAD:WP2 - PAD]
        if spread:
            # Pipeline head: fan the three casts across engines so the
            # first chunk isn't gated on a serial Pool queue.  K goes to
            # DVE (idle at the head; Pool is busy with pad memsets).
            nc.vector.tensor_copy(out=kdst, in_=kst)
            nc.scalar.copy(out=qbf[:, rs, :], in_=qst)
            nc.gpsimd.tensor_copy(out=vdst, in_=vst)
        else:
            nc.gpsimd.tensor_copy(out=kdst, in_=kst)
            nc.gpsimd.tensor_copy(out=vdst, in_=vst)
            nc.scalar.copy(out=qbf[:, rs, :], in_=qst)

    def stage_a(b, ci):
        """QK products -> per-head score matmuls -> exp. Returns E tile."""
        qbf, kpad, vpad = imgs[b]
        y0 = ci * R
        # Hardware caps TensorTensor at 3 free dims: one product
        # instruction per dy group ([dx, R, W]).
        Pg = egrp.tile([128, KK, R, W], BF16, name="Pg", tag="Pg")
        Et = epool.tile([128, KK, R, W], BF16, name="Et", tag="Et")
        qv = _bcast_q(qbf, y0)
        for g in range(KS):  # g == idy
            nc.vector.tensor_mul(Pg[:, g * KS:(g + 1) * KS], qv,
                                 _shift_view(kpad, y0, g))
            Sgt = psS.tile([128, KS, R, W], F32, name="Sgt", tag="Sgt")
            # Planes 0-1 are one bank-aligned 2KB PSUM window: one N=512 MM.
            nc.tensor.matmul(Sgt[:, 0:2], blockones, Pg[:, g * KS:g * KS + 2],
                             start=True, stop=True)
            nc.tensor.matmul(Sgt[:, 2], blockones, Pg[:, g * KS + 2],
                             start=True, stop=True)
            nc.scalar.activation(out=Et[:, g * KS:(g + 1) * KS], in_=Sgt,
                                 func=mybir.ActivationFunctionType.Exp,
                                 scale=SCALE)
        return Et

    state = {"outs": None}

    def stage_b1(b, ci, Et):
        """Z denominator matmuls + AV products."""
        qbf, kpad, vpad = imgs[b]
        y0 = ci * R
        zo = psZ.tile([128, 2, NC], F32, name="zo")
        Zp = zo[:, 0]
        for kk in range(KK):
            nc.tensor.matmul(Zp, bo32, Et[:, kk],
                             start=(kk == 0), stop=(kk == KK - 1))
        Zsb = work.tile([128, NC], F32, name="Zsb", tag="Zsb")
        nc.vector.reciprocal_approx_fast(out=Zsb, in_=Zp)
        P2t = egrp.tile([128, KK, R, W], BF16, name="P2t", tag="P2t")
        for g in range(2):
            nc.vector.tensor_mul(P2t[:, g * KS:(g + 1) * KS],
                                 Et[:, g * KS:(g + 1) * KS],
                                 _shift_view(vpad, y0, g))
        nc.gpsimd.tensor_mul(P2t[:, 2 * KS:], Et[:, 2 * KS:],
                             _shift_view(vpad, y0, KS - 1))
        return zo, Zsb, P2t

    def stage_b(b, ci, bstate):
        """kk-sum, normalize, store."""
        zo, Zsb, P2t = bstate
        Oacc = zo[:, 1]
        for kk in range(KK):
            nc.tensor.matmul(Oacc, identb, P2t[:, kk],
                             start=(kk == 0), stop=(kk == KK - 1))
        outn = work.tile([128, NC], BF16, name="outn")
        # DVE may read only one PSUM operand, and divide is not a hw ALU op:
        # Zsb holds 1/Z (reciprocal_approx_fast), normalize is a multiply.
        nc.vector.tensor_mul(outn, Oacc, Zsb)
        # Channel-major bf16 store; the host gather transposes to [B,H,W,D].
        nc.sync.dma_start(out=out_flat[b][:, ci * NC:(ci + 1) * NC],
                          in_=outn)

    # Dripped prep schedule: image b's load package is spread over the
    # 8 chunk slots preceding its first chunk (image 0's slots land before
    # the loop).  Steps: 0: alloc+memsets+dma band0, 1: dma band1,
    # 2: cast band0 + dma band2, 3: cast band1 + dma band3,
    # 4: cast band2, 5: cast band3.
    def prep_step(b, s):
        if b >= BLOC:
            return
        if b == 0:
            if s == 0:
                prep_alloc(b, dma_first=True)
            elif s == 1:
                prep_dma(b, 1, kq_only=True)
                prep_dma_v(b, 0)
                prep_dma_v(b, 1)
                prep_dma(b, 2)
            elif s == 2:
                prep_cast(b, 0, spread=True)
                prep_cast(b, 1, spread=True)
            elif s in (3, 4):
                prep_dma(b, s)
                prep_cast(b, s - 1)
            elif s == 5:
                prep_cast(b, 4)
            return
        if s == 0:
            prep_alloc(b, dma_first=True)
        elif s in (1, 2, 3, 4):
            prep_cast(b, s - 1)
            prep_dma(b, s)
        elif s == 5:
            prep_cast(b, 4)

    PREP_LEAD = 8   # chunk slots before image start carrying prep steps
    NSTEP = 6

    # Image 0 ramp: bands 0-1 land before the loop; bands 2-4 are
    # dripped into the first chunk iterations (prep_step(0, ...) below)
    # so chunk-0 Pool work isn't queued behind 10 cast instructions.
    prep_step(0, 0)
    emit_consts()
    prep_step(0, 1)
    prep_step(0, 2)

    DEPTH = 6
    B0_RAMP = {0: 3, 2: 4, 4: 5}   # chunk -> image-0 prep step
    tasks = [(b, ci) for b in range(BLOC) for ci in range(NCHUNK)]
    pend = []
    for ti, (b, ci) in enumerate(tasks):
        if b == 0 and ci in B0_RAMP:
            prep_step(0, B0_RAMP[ci])
        # drip next image's prep
        nxt = b + 1 if ci >= NCHUNK - PREP_LEAD else None
        if nxt is not None:
            s = ci - (NCHUNK - PREP_LEAD)
            if s < NSTEP:
                prep_step(nxt, s)
        Eg = stage_a(b, ci)
        pend.append((b, ci, Eg))
        if len(pend) > DEPTH:
            pb, pci, pEg = pend.pop(0)
            stage_b(pb, pci, stage_b1(pb, pci, pEg))
    flshed = [(pb, pci, stage_b1(pb, pci, pEg)) for pb, pci, pEg in pend]
    for pb, pci, bst in flshed:
        stage_b(pb, pci, bst)


_CACHE = {}


def _build():
    if "nc" not in _CACHE:
        nc = bacc.Bacc("TRN2", target_bir_lowering=False, debug=False,
                       num_devices=NCORES)
        q = nc.dram_tensor("q", [BLOC, D, H, W], F32, kind="ExternalInput").ap()
        k = nc.dram_tensor("k", [BLOC, D, H, W], F32, kind="ExternalInput").ap()
        v = nc.dram_tensor("v", [BLOC, D, H, W], F32, kind="ExternalInput").ap()
        out = nc.dram_tensor("out", [BLOC, D, H, W], BF16,
                             kind="ExternalOutput").ap()
        with tile.TileContext(nc) as tc:
            with ExitStack() as ctx:
                _body(ctx, tc, out, q, k, v)
        nc.compile()
        _CACHE["nc"] = nc
    return _CACHE["nc"]


def kernel(q, k, v):
    q = np.ascontiguousarray(np.asarray(q), dtype=np.float32)
    k = np.ascontiguousarray(np.asarray(k), dtype=np.float32)
    v = np.ascontiguousarray(np.asarray(v), dtype=np.float32)
    nc = _build()
    in_maps = [
        {
            "q": np.ascontiguousarray(q[i * BLOC:(i + 1) * BLOC]),
            "k": np.ascontiguousarray(k[i * BLOC:(i + 1) * BLOC]),
            "v": np.ascontiguousarray(v[i * BLOC:(i + 1) * BLOC]),
        }
        for i in range(NCORES)
    ]
    res = run_bass_kernel_spmd(nc, in_maps, list(range(NCORES)),
                               trace=PROFILE)
    # Device output is channel-major bf16 [BLOC, D, H, W]; finish the
    # layout transform and upcast on the host during the gather.
    out = np.concatenate(
        [np.asarray(r["out"]).astype(np.float32).transpose(0, 2, 3, 1)
         for r in res.results], axis=0)
    out = np.ascontiguousarray(out)
    if PROFILE:
        kernel.last_exec_time_ns = res.exec_time_ns
        kernel.last_results = res
    return out


if __name__ == "__main__":
    nc = _build()
    print("build OK")
    from concourse.timeline_sim import TimelineSim
    tl = TimelineSim(nc, trace=False)
    t = tl.simulate()
    print(f"TimelineSim: {t/1000.0:.1f} us")
